# revision 10
# baseline (speedup 1.0000x reference)
"""Trainium2 Bass kernel for nn_CBPoolMax2d.

Reference semantics: changeIndexes are flat spatial indices (y*W+x) of changed
input pixels; each maps to output pixel (y//2, x//2).  The output is the
persistent outputState with the 2x2-max-pooled value recomputed at every
changed output pixel (all channels).

Equivalent dense formulation used here:
    out = where(mask, maxpool2x2(input), outputState)
where mask[oy, ox] = any changeIndex maps to (oy, ox).  The mask is built on
host from the 128 KB index vector; all heavy data (input 256 MB, state 64 MB)
streams through the 8 NeuronCores, sharded over the channel dim (32 ch/core).

Per-core device kernel (all f32):
  partitions = (channel, row-block): P = 32ch x 4rb = 128
  for each of 8 row-tiles (64 input rows):
    DMA input tile  [128, 16*512]
    hmax = max over col pairs   (DVE tensor_tensor, strided)
    vmax = max over row pairs   (DVE tensor_tensor, strided)
    DMA state tile -> out tile [128, 8*256]
    DMA mask tile  [128, 8*256]
    copy_predicated(out, mask, vmax)
    DMA out tile -> out DRAM
"""

import os
import numpy as np

C, H, W = 256, 512, 512
OH, OW = H // 2, W // 2
NCORES = 8
CPC = C // NCORES          # 32 channels per core

P = 128                    # SBUF partitions = (channel, row-block)
RB = P // CPC              # 4 row-blocks
NT = 8                     # row tiles
ROWS_PER_TILE = H // NT    # 64 input rows per tile
R = ROWS_PER_TILE // RB    # 16 input rows per partition per tile
FREE_IN = R * W            # 8192
ORPP = R // 2              # 8 output rows per partition per tile
FREE_OUT = ORPP * OW       # 2048

TRACE = os.environ.get("CBPOOL_TRACE", "0") == "1"
last_results = None

_cache = {}


def _build_nc():
    import concourse.bacc as bacc
    import concourse.tile as tile
    from concourse import bass, mybir

    f32 = mybir.dt.float32
    nc = bacc.Bacc("TRN2", target_bir_lowering=False, debug=False,
                   num_devices=NCORES)
    u8 = mybir.dt.uint8
    inp = nc.dram_tensor("inp", [CPC, H, W], f32, kind="ExternalInput")
    state = nc.dram_tensor("state", [CPC, OH, OW], f32, kind="ExternalInput")
    mask = nc.dram_tensor("mask", [CPC, OH, OW], u8, kind="ExternalInput")
    out = nc.dram_tensor("out", [CPC, OH, OW], f32, kind="ExternalOutput")

    with tile.TileContext(nc) as tc:
        with tc.tile_pool(name="pin", bufs=3) as pin, \
             tc.tile_pool(name="ph", bufs=2) as ph, \
             tc.tile_pool(name="pv", bufs=2) as pv, \
             tc.tile_pool(name="pm", bufs=3) as pm, \
             tc.tile_pool(name="po", bufs=3) as po:
            for t in range(NT):
                in_t = pin.tile([P, FREE_IN], f32)
                src = bass.AP(inp, t * ROWS_PER_TILE * W,
                              [[H * W, CPC], [R * W, RB], [1, FREE_IN]])
                nc.sync.dma_start(in_t[:], src)

                # hmax over column pairs: [P, R, OW]
                h_t = ph.tile([P, R * OW], f32)
                in_v = in_t[:].rearrange("p (r x w) -> p r x w",
                                         r=R, x=OW, w=2)
                h_v = h_t[:].rearrange("p (r x) -> p r x", r=R, x=OW)
                nc.vector.tensor_tensor(out=h_v, in0=in_v[:, :, :, 0],
                                        in1=in_v[:, :, :, 1],
                                        op=mybir.AluOpType.max)

                # vmax over row pairs: [P, ORPP, OW]
                v_t = pv.tile([P, FREE_OUT], f32)
                h_vv = h_t[:].rearrange("p (r2 wr x) -> p r2 x wr",
                                        r2=ORPP, wr=2, x=OW)
                v_v = v_t[:].rearrange("p (r2 x) -> p r2 x", r2=ORPP, x=OW)
                nc.vector.tensor_tensor(out=v_v, in0=h_vv[:, :, :, 0],
                                        in1=h_vv[:, :, :, 1],
                                        op=mybir.AluOpType.max)

                # out tile starts as the state slice; overlay pooled where mask
                st_pat = [[OH * OW, CPC], [ORPP * OW, RB], [1, FREE_OUT]]
                st_off = t * ORPP * RB * OW
                out_t = po.tile([P, FREE_OUT], f32)
                nc.scalar.dma_start(out_t[:], bass.AP(state, st_off, st_pat))
                m_t = pm.tile([P, FREE_OUT], u8)
                nc.scalar.dma_start(m_t[:], bass.AP(mask, st_off, st_pat))
                nc.vector.copy_predicated(out=out_t[:], mask=m_t[:],
                                          data=v_t[:])

                nc.scalar.dma_start(bass.AP(out, st_off, st_pat), out_t[:])

    nc.compile()
    return nc


def _get_nc():
    if "nc" not in _cache:
        _cache["nc"] = _build_nc()
    return _cache["nc"]


def kernel(input, outputState, changeIndexes):
    global last_results
    from concourse.bass_utils import run_bass_kernel_spmd

    nc = _get_nc()

    inp = np.ascontiguousarray(
        np.asarray(input, dtype=np.float32).reshape(C, H, W))
    state = np.ascontiguousarray(
        np.asarray(outputState, dtype=np.float32).reshape(C, OH, OW))
    ci = np.asarray(changeIndexes).astype(np.int64)

    oy = (ci // W) // 2
    ox = (ci % W) // 2
    mask = np.zeros((OH, OW), dtype=np.uint8)
    mask[oy, ox] = 1
    mask_rep = np.broadcast_to(mask[None], (CPC, OH, OW))

    in_maps = [
        {
            "inp": inp[i * CPC:(i + 1) * CPC],
            "state": state[i * CPC:(i + 1) * CPC],
            "mask": mask_rep,
        }
        for i in range(NCORES)
    ]
    res = run_bass_kernel_spmd(nc, in_maps, core_ids=list(range(NCORES)),
                               trace=TRACE)
    last_results = res
    out = np.concatenate([res.results[i]["out"] for i in range(NCORES)],
                         axis=0)
    return out.reshape(1, C, OH, OW).astype(np.float32, copy=False)


# revision 18
# speedup vs baseline: 1.1901x; 1.1901x over previous
"""Trainium2 Bass kernel for nn_CBPoolMax2d.

Reference semantics: changeIndexes are flat spatial indices (y*W+x) of changed
input pixels; each maps to output pixel (y//2, x//2).  The output is the
persistent outputState with the 2x2-max-pooled value recomputed at every
changed output pixel (all channels).

Equivalent dense formulation used here:
    out = where(mask, maxpool2x2(input), outputState)
where mask[oy, ox] = any changeIndex maps to (oy, ox).  The mask is built on
host from the 128 KB index vector; all heavy data (input 256 MB, state 64 MB)
streams through the 8 NeuronCores, sharded over the channel dim (32 ch/core).

Per-core device kernel:
  partitions = (channel, row-block): P = 32ch x 4rb = 128
  for each of 8 row-tiles (64 input rows):
    DMA input tile [128, 16*512] f32        (sync HWDGE ring)
    hmax = max over col pairs               (DVE tensor_tensor, strided)
    vmax = max over row pairs               (DVE tensor_tensor, strided)
    DMA state tile -> out tile [128, 8*256] (scalar HWDGE ring)
    DMA uint8 mask tile [128, 8*256]        (scalar HWDGE ring)
    copy_predicated(out, mask, vmax)        (DVE)
    DMA out tile -> out DRAM                (scalar HWDGE ring)

This streams 48 MB of f32 payload + 2 MB mask per core; measured HW exec
~144 us, at the per-HBM-stack roofline (two cores share a 716 GB/s stack:
2 x 48 MB / 716 GB/s = 134 us body + ~11 us fixed NEFF barrier overhead).
"""

import os
import numpy as np

C, H, W = 256, 512, 512
OH, OW = H // 2, W // 2
NCORES = 8
CPC = C // NCORES          # 32 channels per core

P = 128                    # SBUF partitions = (channel, row-block)
RB = P // CPC              # 4 row-blocks
NT = 8                     # row tiles
ROWS_PER_TILE = H // NT    # 64 input rows per tile
R = ROWS_PER_TILE // RB    # 16 input rows per partition per tile
FREE_IN = R * W            # 8192
ORPP = R // 2              # 8 output rows per partition per tile
FREE_OUT = ORPP * OW       # 2048

TRACE = os.environ.get("CBPOOL_TRACE", "0") == "1"
last_results = None

_cache = {}


def _build_nc():
    import concourse.bacc as bacc
    import concourse.tile as tile
    from concourse import bass, mybir

    f32 = mybir.dt.float32
    nc = bacc.Bacc("TRN2", target_bir_lowering=False, debug=False,
                   num_devices=NCORES)
    u8 = mybir.dt.uint8
    inp = nc.dram_tensor("inp", [CPC, H, W], f32, kind="ExternalInput")
    state = nc.dram_tensor("state", [CPC, OH, OW], f32, kind="ExternalInput")
    mask = nc.dram_tensor("mask", [CPC, OH, OW], u8, kind="ExternalInput")
    out = nc.dram_tensor("out", [CPC, OH, OW], f32, kind="ExternalOutput")

    with tile.TileContext(nc) as tc:
        with tc.tile_pool(name="pin", bufs=3) as pin, \
             tc.tile_pool(name="ph", bufs=2) as ph, \
             tc.tile_pool(name="pv", bufs=2) as pv, \
             tc.tile_pool(name="pm", bufs=3) as pm, \
             tc.tile_pool(name="po", bufs=3) as po:
            for t in range(NT):
                in_t = pin.tile([P, FREE_IN], f32)
                src = bass.AP(inp, t * ROWS_PER_TILE * W,
                              [[H * W, CPC], [R * W, RB], [1, FREE_IN]])
                nc.sync.dma_start(in_t[:], src)

                # hmax over column pairs: [P, R, OW]
                h_t = ph.tile([P, R * OW], f32)
                in_v = in_t[:].rearrange("p (r x w) -> p r x w",
                                         r=R, x=OW, w=2)
                h_v = h_t[:].rearrange("p (r x) -> p r x", r=R, x=OW)
                nc.vector.tensor_tensor(out=h_v, in0=in_v[:, :, :, 0],
                                        in1=in_v[:, :, :, 1],
                                        op=mybir.AluOpType.max)

                # vmax over row pairs: [P, ORPP, OW]
                v_t = pv.tile([P, FREE_OUT], f32)
                h_vv = h_t[:].rearrange("p (r2 wr x) -> p r2 x wr",
                                        r2=ORPP, wr=2, x=OW)
                v_v = v_t[:].rearrange("p (r2 x) -> p r2 x", r2=ORPP, x=OW)
                nc.vector.tensor_tensor(out=v_v, in0=h_vv[:, :, :, 0],
                                        in1=h_vv[:, :, :, 1],
                                        op=mybir.AluOpType.max)

                # out tile starts as the state slice; overlay pooled where mask
                st_pat = [[OH * OW, CPC], [ORPP * OW, RB], [1, FREE_OUT]]
                st_off = t * ORPP * RB * OW
                out_t = po.tile([P, FREE_OUT], f32)
                nc.scalar.dma_start(out_t[:], bass.AP(state, st_off, st_pat))

                m_t = pm.tile([P, FREE_OUT], u8)
                nc.scalar.dma_start(m_t[:], bass.AP(mask, st_off, st_pat))
                nc.vector.copy_predicated(out=out_t[:], mask=m_t[:],
                                          data=v_t[:])

                nc.scalar.dma_start(bass.AP(out, st_off, st_pat), out_t[:])

    nc.compile()
    return nc


def _get_nc():
    if "nc" not in _cache:
        _cache["nc"] = _build_nc()
    return _cache["nc"]


def kernel(input, outputState, changeIndexes):
    global last_results
    from concourse.bass_utils import run_bass_kernel_spmd

    nc = _get_nc()

    inp = np.ascontiguousarray(
        np.asarray(input, dtype=np.float32).reshape(C, H, W))
    state = np.ascontiguousarray(
        np.asarray(outputState, dtype=np.float32).reshape(C, OH, OW))
    ci = np.asarray(changeIndexes).astype(np.int64)

    oy = (ci // W) // 2
    ox = (ci % W) // 2
    mask = np.zeros((OH, OW), dtype=np.uint8)
    mask[oy, ox] = 1
    mask_rep = np.broadcast_to(mask[None], (CPC, OH, OW))

    in_maps = [
        {
            "inp": inp[i * CPC:(i + 1) * CPC],
            "state": state[i * CPC:(i + 1) * CPC],
            "mask": mask_rep,
        }
        for i in range(NCORES)
    ]
    res = run_bass_kernel_spmd(nc, in_maps, core_ids=list(range(NCORES)),
                               trace=TRACE)
    last_results = res
    out = np.concatenate([res.results[i]["out"] for i in range(NCORES)],
                         axis=0)
    return out.reshape(1, C, OH, OW).astype(np.float32, copy=False)
